# revision 31
# baseline (speedup 1.0000x reference)
"""Trainium2 Bass kernel for a 2-layer GCN (nn_EvenLamerGCN).

reference semantics (PyG GCNConv x2, eval mode):
    deg[i]  = 1 + indeg(i)                (self-loops added)
    dinv    = deg ** -0.5
    h  = relu(A_hat @ (x @ W1) + b1),  A_hat = D^-1/2 (A + I) D^-1/2
    o  = A_hat @ (h @ W2) + b2
    return o, log_softmax(o, axis=1)

Distribution: nodes sharded over 8 NeuronCores (12500/core, padded to
12544), edges partitioned by destination core.  The per-edge norm is
folded into per-node row scalings:
    out = dinv * ( sum_{e: dst=i} T[src_e] + T[i] ),   T = dinv * (x @ W)

Gather tables are stored seg-interleaved ([seg][core][rows]) so that a
src window == one seg-block and each window becomes gatherable right
after its own segmented AllGather:
  p0 computes T per node-block, seeds the self-loop term directly into
  the accumulator, and emits one AllGather per seg.  Edge aggregation
  (dma_gather of 256B rows + one-hot matmul segment-sum) streams per
  window; the last window is processed in dst-block seg groups with the
  layer-2 table matmul + segmented AllGather2 pipelined into the tail so
  layer-2 gathers start with minimal bubble.  SPMD: one NEFF, identical
  instruction streams; per-core variation lives in input data.
"""

import sys

for _p in ("/opt/trn_rl_repo", "/root/.axon_site/_ro/trn_rl_repo"):
    if _p not in sys.path:
        sys.path.insert(0, _p)

from contextlib import ExitStack
from dataclasses import dataclass

import numpy as np

import concourse.bass as bass
import concourse.mybir as mybir
import concourse.tile as tile
from concourse import bacc
from concourse.bass import ds, ts
from concourse.bass_utils import run_bass_kernel_spmd
from concourse.masks import make_identity

F32 = mybir.dt.float32
BF16 = mybir.dt.bfloat16
I16 = mybir.dt.int16
AF = mybir.ActivationFunctionType
ALU = mybir.AluOpType


@dataclass(frozen=True)
class Cfg:
    n: int = 100000          # nodes
    din: int = 512           # input features
    dh: int = 128            # hidden features
    dout: int = 40           # output features
    cores: int = 8
    segb: tuple = (25, 25, 24, 24)   # dst/src blocks per seg (per core)
    max_piece: int = 16      # chunks per gather instruction
    dma_scratch: int = 32768  # SWDGE descriptor carveout (bytes; ndesc = /16)

    @property
    def nsh(self):           # real nodes per core
        return self.n // self.cores

    @property
    def nloc(self):          # padded nodes per core (multiple of 128)
        return ((self.nsh + 127) // 128) * 128

    @property
    def nt(self):            # 128-node blocks per core
        return self.nloc // 128

    @property
    def nwin(self):
        return len(self.segb)

    @property
    def seg_start_block(self):
        return tuple(int(x) for x in np.cumsum((0,) + self.segb[:-1]))

    @property
    def seg_rows(self):      # rows per core per seg
        return tuple(128 * b for b in self.segb)

    @property
    def wsizes(self):        # gather window sizes (= seg-block rows)
        return tuple(self.cores * r for r in self.seg_rows)

    @property
    def wbases(self):
        return tuple(int(x) for x in np.cumsum((0,) + self.wsizes[:-1]))

    @property
    def trows(self):
        return self.cores * self.nloc

    @property
    def dh2(self):           # padded output width
        return max(64, ((self.dout + 63) // 64) * 64)

    @property
    def dt2(self):           # layer-2 bf16 table row width (256B rows)
        return max(128, self.dh2)

    @property
    def kt(self):            # k-tiles in the first matmul
        return self.din // 128


@dataclass(frozen=True)
class Plan:
    quotas: tuple          # chunks per (block, window) cell
    sections: tuple        # w0..w2: piece sizes; w3: per-group piece sizes

    @property
    def total_chunks(self):
        tot = 0
        for w, s in enumerate(self.sections):
            if isinstance(s[0], tuple):
                tot += sum(sum(g) for g in s)
            else:
                tot += sum(s)
        return tot


def _pieces(nchunks, max_piece):
    sizes = []
    while nchunks > 0:
        sizes.append(min(max_piece, nchunks))
        nchunks -= sizes[-1]
    return tuple(sizes)


# ----------------------------------------------------------------------------
# CPU-side preprocessing
# ----------------------------------------------------------------------------

def preprocess(cfg: Cfg, edge_index: np.ndarray):
    c = cfg
    src = np.asarray(edge_index[0], dtype=np.int64)
    dst = np.asarray(edge_index[1], dtype=np.int64)

    deg = np.bincount(dst, minlength=c.n).astype(np.float32) + 1.0
    deg_pt = np.ones((c.cores, 128, c.nt), np.float32)
    for ci in range(c.cores):
        dl = np.ones(c.nloc, np.float32)
        dl[: c.nsh] = deg[ci * c.nsh : (ci + 1) * c.nsh]
        deg_pt[ci] = dl.reshape(c.nt, 128).T

    seg_start_loc = np.array([128 * b for b in c.seg_start_block], np.int64)
    seg_rows = np.array(c.seg_rows, np.int64)
    wbases = np.array(c.wbases, np.int64)

    # seg-interleaved table row of global node i
    core_s = src // c.nsh
    loc_s = src % c.nsh
    seg_s = np.searchsorted(seg_start_loc, loc_s, side="right") - 1
    r_all = wbases[seg_s] + core_s * seg_rows[seg_s] + (loc_s - seg_start_loc[seg_s])
    w_all = seg_s

    core_all = dst // c.nsh
    dloc_all = dst - core_all * c.nsh
    b_all = dloc_all // 128
    id_all = dloc_all % 128

    # count edges per (core, block, window) -> uniform chunk quotas
    cell_key = (core_all * c.nt + b_all) * c.nwin + w_all
    counts = np.bincount(cell_key, minlength=c.cores * c.nt * c.nwin)
    counts = counts.reshape(c.cores, c.nt, c.nwin)
    quotas = tuple(int(-(-counts[:, :, w].max() // 128)) for w in range(c.nwin))

    sections = []
    for w in range(c.nwin - 1):
        sections.append(_pieces(c.nt * quotas[w], c.max_piece))
    qlast = quotas[c.nwin - 1]
    sections.append(tuple(_pieces(b * qlast, c.max_piece) for b in c.segb))
    plan = Plan(quotas=quotas, sections=tuple(sections))

    total_chunks = plan.total_chunks
    slots = total_chunks * 128

    idx16 = np.zeros((c.cores, 128, slots // 16), np.int16)
    ids_f32 = np.empty((c.cores, 128, total_chunks), np.float32)

    order = np.lexsort((r_all, w_all, b_all, core_all))
    so_r, so_w, so_b, so_core, so_id = (
        r_all[order], w_all[order], b_all[order], core_all[order], id_all[order]
    )
    core_starts = np.searchsorted(so_core, np.arange(c.cores + 1))

    for ci in range(c.cores):
        lo, hi = core_starts[ci], core_starts[ci + 1]
        rr, ii = so_r[lo:hi], so_id[lo:hi]
        rel = np.zeros(slots, np.int64)      # window-relative gather rows
        ids = np.full(slots, -1.0, np.float32)
        sec_off = np.cumsum([0] + [c.nt * q * 128 for q in quotas])
        pos = 0
        # sorted order within a core is (b, w, r); cells land at
        # sec_off[w] + b * quotas[w] * 128
        for b in range(c.nt):
            for w in range(c.nwin):
                cnt = counts[ci, b, w]
                if cnt:
                    off = sec_off[w] + b * quotas[w] * 128
                    rel[off : off + cnt] = rr[pos : pos + cnt] - c.wbases[w]
                    assert rel[off : off + cnt].max() < c.wsizes[w]
                    ids[off : off + cnt] = ii[pos : pos + cnt]
                    pos += cnt
        assert pos == hi - lo
        assert rel.min() >= 0

        v = rel.reshape(-1, 16)              # slot i at [i%16, i//16]
        wrapped = np.ascontiguousarray(v.T)  # [16, slots/16]
        idx16[ci] = np.tile(wrapped, (8, 1)).astype(np.int16)
        ids_f32[ci] = ids.reshape(total_chunks, 128).T

    return deg_pt, idx16, ids_f32, plan


# ----------------------------------------------------------------------------
# Device kernel
# ----------------------------------------------------------------------------

def build(nc, tc, cfg: Cfg, plan: Plan):
    c = cfg
    RG = [list(range(c.cores))]
    total_chunks = plan.total_chunks
    slots = total_chunks * 128
    nseg = c.nwin
    ssb = c.seg_start_block

    x_sh = nc.dram_tensor("x_sh", [c.nloc, c.din], BF16, kind="ExternalInput").ap()
    w1 = nc.dram_tensor("w1", [c.din, c.dh], BF16, kind="ExternalInput").ap()
    w2 = nc.dram_tensor("w2", [c.dh, c.dh2], F32, kind="ExternalInput").ap()
    b1r = nc.dram_tensor("b1r", [128, c.dh], F32, kind="ExternalInput").ap()
    b2r = nc.dram_tensor("b2r", [128, c.dh2], F32, kind="ExternalInput").ap()
    degp = nc.dram_tensor("degp", [128, c.nt], F32, kind="ExternalInput").ap()
    idx16 = nc.dram_tensor("idx16", [128, slots // 16], I16, kind="ExternalInput").ap()
    idsf = nc.dram_tensor("idsf", [128, total_chunks], BF16, kind="ExternalInput").ap()
    out_h = nc.dram_tensor("out_h", [c.nloc, c.dh2], F32, kind="ExternalOutput").ap()
    out_ls = nc.dram_tensor("out_ls", [c.nloc, c.dh2], F32, kind="ExternalOutput").ap()

    # per-seg local table shards and per-window (seg-block) gathered tables
    t1_loc = [
        nc.dram_tensor(f"t1_loc{s}", [c.seg_rows[s], c.dh], BF16, kind="Internal").ap()
        for s in range(nseg)
    ]
    t1_full = [
        nc.dram_tensor(
            f"t1_full{s}", [c.wsizes[s], c.dh], BF16, kind="Internal",
            addr_space="Shared",
        ).ap()
        for s in range(nseg)
    ]
    t2_loc = [
        nc.dram_tensor(f"t2_loc{s}", [c.seg_rows[s], c.dt2], BF16, kind="Internal").ap()
        for s in range(nseg)
    ]
    t2_full = [
        nc.dram_tensor(
            f"t2_full{s}", [c.wsizes[s], c.dt2], BF16, kind="Internal",
            addr_space="Shared",
        ).ap()
        for s in range(nseg)
    ]

    with ExitStack() as st:
        cpool = st.enter_context(tc.tile_pool(name="consts", bufs=1))
        accp = st.enter_context(tc.tile_pool(name="acc", bufs=1))
        gp = st.enter_context(tc.tile_pool(name="gp", bufs=4))
        sp = st.enter_context(tc.tile_pool(name="sp", bufs=2))
        pp = st.enter_context(tc.tile_pool(name="pp", bufs=3))
        ppsum = st.enter_context(tc.tile_pool(name="ppsum", bufs=4, space="PSUM"))
        p0 = st.enter_context(tc.tile_pool(name="p0", bufs=3))
        p0ps = st.enter_context(tc.tile_pool(name="p0ps", bufs=2, space="PSUM"))
        p0psT = st.enter_context(tc.tile_pool(name="p0psT", bufs=2, space="PSUM"))

        # ---- constants ----
        ident = cpool.tile([128, 128], F32)
        make_identity(nc, ident)
        identb = cpool.tile([128, 128], BF16)
        make_identity(nc, identb)
        w1sb = cpool.tile([128, c.kt, c.dh], BF16)
        nc.sync.dma_start(w1sb, w1.rearrange("(o p) f -> p o f", p=128))
        w2sb = cpool.tile([128, c.dh2], F32)
        nc.sync.dma_start(w2sb, w2)
        b1sb = cpool.tile([128, c.dh], F32)
        nc.sync.dma_start(b1sb, b1r)
        b2sb = cpool.tile([128, c.dh2], F32)
        nc.sync.dma_start(b2sb, b2r)
        dinv = cpool.tile([128, c.nt], F32)
        nc.sync.dma_start(dinv, degp)
        nc.scalar.activation(dinv, dinv, AF.Sqrt)
        nc.vector.reciprocal(dinv, dinv)
        iota = cpool.tile([128, c.max_piece, 128], BF16)
        nc.gpsimd.iota(iota, pattern=[[0, c.max_piece], [1, 128]], base=0,
                       channel_multiplier=0,
                       allow_small_or_imprecise_dtypes=True)

        acc1 = [accp.tile([128, c.segb[s], c.dh], F32, tag=f"a1_{s}", name=f"acc1_{s}")
                for s in range(nseg)]
        acc2 = accp.tile([128, c.nt, c.dh2], F32, tag="a2")

        # ---- phase 0: T1 = dinv * (x @ W1); seed self-loop; seg AllGathers --
        with nc.named_scope("p0_mm1"):
            for s in range(nseg):
                for j in range(c.segb[s]):
                    t = ssb[s] + j
                    xt = p0.tile([128, c.din], BF16, tag="xt")
                    nc.sync.dma_start(xt, x_sh[ts(t, 128), :])
                    hps = p0ps.tile([128, c.dh], F32, tag="hps")
                    for k in range(c.kt):
                        tps = p0psT.tile([128, 128], BF16, tag="tps")
                        nc.tensor.transpose(tps, xt[:, ts(k, 128)], identb)
                        xT = p0.tile([128, 128], BF16, tag="xT")
                        nc.vector.tensor_copy(xT, tps)
                        nc.tensor.matmul(
                            hps, lhsT=xT, rhs=w1sb[:, k, :],
                            start=(k == 0), stop=(k == c.kt - 1),
                        )
                    hsb = p0.tile([128, c.dh], BF16, tag="hsb")
                    nc.vector.tensor_scalar_mul(hsb, hps, dinv[:, t : t + 1])
                    nc.vector.tensor_copy(acc1[s][:, j, :], hsb)  # self-loop seed
                    nc.sync.dma_start(t1_loc[s][ts(j, 128), :], hsb)
                nc.gpsimd.collective_compute(
                    "AllGather", ALU.bypass, replica_groups=RG,
                    ins=[t1_loc[s].opt()], outs=[t1_full[s].opt()],
                )

        # ---- edge gather + segment-sum machinery ----
        state = {"chunk0": 0, "piece": 0}

        def gather_pieces(tables, accs, d, dt, sizes, w, b0, sit, sid, loc0,
                          seed_from=None):
            """Emit gather pieces covering `sizes` chunks of window w starting
            at dst block b0; PSUM-accumulate per block into accs."""
            q = plan.quotas[w]
            loc = loc0
            k_in_block = 0
            b = b0
            ps = None
            for nch in sizes:
                g = gp.tile([128, c.max_piece, dt], BF16, tag="gt")
                qn = state["piece"] % 4
                state["piece"] += 1
                nc.gpsimd.dma_gather(
                    g[:, :nch, :], tables[w],
                    sit[:, loc * 8 : (loc + nch) * 8],
                    num_idxs=nch * 128, num_idxs_reg=nch * 128, elem_size=dt,
                    single_packet=False, queue_num=qn,
                )
                stt = pp.tile([128, c.max_piece, 128], BF16, tag="stt")
                nc.vector.tensor_tensor(
                    stt[:, :nch, :], iota[:, :nch, :],
                    sid[:, loc : loc + nch, None].to_broadcast((128, nch, 128)),
                    ALU.is_equal,
                )
                for j in range(nch):
                    if k_in_block == 0:
                        ps = ppsum.tile([128, d], F32, tag="ps")
                    nc.tensor.matmul(
                        ps, lhsT=stt[:, j, :], rhs=g[:, j, :d],
                        start=(k_in_block == 0), stop=(k_in_block == q - 1),
                    )
                    k_in_block += 1
                    if k_in_block == q:
                        seg = int(np.searchsorted(ssb, b, side="right")) - 1
                        acc_t = accs[seg] if isinstance(accs, list) else accs
                        jb = b - ssb[seg] if isinstance(accs, list) else b
                        nc.vector.tensor_tensor(
                            acc_t[:, jb, :], acc_t[:, jb, :], ps, ALU.add
                        )
                        b += 1
                        k_in_block = 0
                loc += nch
                state["chunk0"] += nch
            assert k_in_block == 0
            return loc, b

        max_sec = max(
            (sum(sum(g) for g in s) if isinstance(s[0], tuple) else sum(s))
            for s in plan.sections
        )

        def load_section(w, sec_ch):
            sit = sp.tile([128, max_sec * 8], I16, tag="sit")
            nc.sync.dma_start(
                sit[:, : sec_ch * 8],
                idx16[:, state["chunk0"] * 8 : (state["chunk0"] + sec_ch) * 8],
            )
            sid = sp.tile([128, max_sec], BF16, tag="sid")
            nc.sync.dma_start(
                sid[:, :sec_ch],
                idsf[:, state["chunk0"] : state["chunk0"] + sec_ch],
            )
            return sit, sid

        def mid_p2_seg(s):
            """acc1_s -> relu/scale, then T2 blocks + seed acc2 + t2_loc."""
            a = acc1[s]
            nb = c.segb[s]
            dv = dinv[:, ssb[s] : ssb[s] + nb, None].to_broadcast((128, nb, c.dh))
            nc.vector.tensor_tensor(a, a, dv, ALU.mult)
            nc.vector.tensor_tensor(
                a, a, b1sb[:, None, :].to_broadcast((128, nb, c.dh)), ALU.add
            )
            nc.scalar.activation(a, a, AF.Relu)
            nc.vector.tensor_tensor(a, a, dv, ALU.mult)
            for j in range(nb):
                tps = p0psT.tile([128, 128], F32, tag="tps")
                nc.tensor.transpose(tps, a[:, j, :], ident)
                gT = p0.tile([128, 128], F32, tag="xT")
                nc.vector.tensor_copy(gT, tps)
                h2ps = p0ps.tile([128, c.dh2], F32, tag="hps")
                nc.tensor.matmul(h2ps, lhsT=gT, rhs=w2sb, start=True, stop=True)
                h2sb = p0.tile([128, c.dh2], BF16, tag="h2sb")
                nc.vector.tensor_copy(h2sb, h2ps)
                nc.vector.tensor_copy(acc2[:, ssb[s] + j, :], h2sb)  # self-loop
                nc.sync.dma_start(t2_loc[s][ts(j, 128), : c.dh2], h2sb)

        # ---- layer-1 edge phase; w3 grouped by dst seg with p2/AG2 woven in --
        with nc.named_scope("edge1"):
            for w in range(nseg - 1):
                sizes = plan.sections[w]
                sit, sid = load_section(w, sum(sizes))
                gather_pieces(t1_full, acc1, c.dh, c.dh, sizes, w, 0, sit, sid, 0)
            wl = nseg - 1
            gsizes = plan.sections[wl]
            sit, sid = load_section(wl, sum(sum(g) for g in gsizes))
            loc = 0
            b = 0
            for g in range(nseg):
                loc, b = gather_pieces(
                    t1_full, acc1, c.dh, c.dh, gsizes[g], wl, b, sit, sid, loc
                )
                if g >= 1:
                    nc.gpsimd.collective_compute(
                        "AllGather", ALU.bypass, replica_groups=RG,
                        ins=[t2_loc[g - 1].opt()], outs=[t2_full[g - 1].opt()],
                    )
                mid_p2_seg(g)
            nc.gpsimd.collective_compute(
                "AllGather", ALU.bypass, replica_groups=RG,
                ins=[t2_loc[nseg - 1].opt()], outs=[t2_full[nseg - 1].opt()],
            )

        # ---- layer-2 edge phase (same slot layout; restart the cursor) ----
        state["chunk0"] = 0
        with nc.named_scope("edge2"):
            for w in range(nseg - 1):
                sizes = plan.sections[w]
                sit, sid = load_section(w, sum(sizes))
                gather_pieces(t2_full, acc2, c.dh2, c.dt2, sizes, w, 0, sit, sid, 0)
            gsizes = plan.sections[wl]
            sit, sid = load_section(wl, sum(sum(g) for g in gsizes))
            loc = 0
            b = 0
            for g in range(nseg):
                loc, b = gather_pieces(
                    t2_full, acc2, c.dh2, c.dt2, gsizes[g], wl, b, sit, sid, loc
                )

        # ---- h = dinv * agg2 + b2 ; log_softmax (batched) ----
        with nc.named_scope("tail"):
            ohv = out_h.rearrange("(t p) f -> p t f", p=128)
            olv = out_ls.rearrange("(t p) f -> p t f", p=128)
            nc.vector.tensor_tensor(
                acc2, acc2, dinv[:, :, None].to_broadcast((128, c.nt, c.dh2)),
                ALU.mult,
            )
            nc.vector.tensor_tensor(
                acc2, acc2, b2sb[:, None, :].to_broadcast((128, c.nt, c.dh2)),
                ALU.add,
            )
            nc.sync.dma_start(ohv, acc2)
            accN = acc2[:, :, : c.dout]
            mx = accp.tile([128, c.nt], F32, tag="mx")
            nc.vector.tensor_reduce(mx, accN, mybir.AxisListType.X, ALU.max)
            nc.vector.tensor_tensor(
                accN, accN, mx[:, :, None].to_broadcast((128, c.nt, c.dout)),
                ALU.subtract,
            )
            e1 = accp.tile([128, c.nt, c.dout], F32, tag="e1")
            nc.scalar.activation(e1, accN, AF.Exp)
            se = accp.tile([128, c.nt], F32, tag="se")
            nc.vector.tensor_reduce(se, e1, mybir.AxisListType.X, ALU.add)
            ln = accp.tile([128, c.nt], F32, tag="ln")
            nc.scalar.activation(ln, se, AF.Ln)
            nc.vector.tensor_tensor(
                accN, accN, ln[:, :, None].to_broadcast((128, c.nt, c.dout)),
                ALU.subtract,
            )
            nc.sync.dma_start(olv[:, :, : c.dout], accN)


# ----------------------------------------------------------------------------
# Host entry point
# ----------------------------------------------------------------------------

_CACHE = {}


def _get_compiled(cfg: Cfg, plan: Plan):
    key = (cfg, plan)
    if key not in _CACHE:
        nc = bacc.Bacc(
            "TRN2", target_bir_lowering=False, debug=False,
            num_devices=cfg.cores, num_swdge_queues=4,
            dynamic_dma_scratch_size=cfg.dma_scratch,
        )
        with tile.TileContext(nc) as tc:
            build(nc, tc, cfg, plan)
        nc.compile()
        _CACHE[key] = nc
    return _CACHE[key]


def make_in_maps(cfg: Cfg, x, W1, b1, W2, b2, deg_pt, idx16, ids_f32):
    import ml_dtypes

    c = cfg
    x = np.asarray(x, np.float32)
    w2p = np.zeros((c.dh, c.dh2), np.float32)
    w2p[:, : c.dout] = np.asarray(W2, np.float32)
    b1rep = np.tile(np.asarray(b1, np.float32)[None, :], (128, 1))
    b2p = np.zeros(c.dh2, np.float32)
    b2p[: c.dout] = np.asarray(b2, np.float32)
    b2rep = np.tile(b2p[None, :], (128, 1))
    w1c = np.ascontiguousarray(
        np.asarray(W1, np.float32).astype(ml_dtypes.bfloat16)
    )

    in_maps = []
    for ci in range(c.cores):
        xs = np.zeros((c.nloc, c.din), ml_dtypes.bfloat16)
        xs[: c.nsh] = x[ci * c.nsh : (ci + 1) * c.nsh].astype(ml_dtypes.bfloat16)
        in_maps.append({
            "x_sh": xs,
            "w1": w1c,
            "w2": w2p,
            "b1r": b1rep,
            "b2r": b2rep,
            "degp": np.ascontiguousarray(deg_pt[ci]),
            "idx16": np.ascontiguousarray(idx16[ci]),
            "idsf": np.ascontiguousarray(ids_f32[ci].astype(ml_dtypes.bfloat16)),
        })
    return in_maps


def _ensure_ntff_hook():
    """Install the axon NTFF profile hook if the image's antenv lacks it."""
    import types

    try:
        from antenv.axon_hooks import get_axon_ntff_profile_hook  # noqa: F401
        return
    except ImportError:
        pass
    import antenv

    m = types.ModuleType("antenv.axon_hooks")
    m._hook = None
    m.set_axon_ntff_profile_hook = lambda h: setattr(m, "_hook", h)
    m.get_axon_ntff_profile_hook = lambda: m._hook
    sys.modules["antenv.axon_hooks"] = m
    antenv.axon_hooks = m
    try:
        from trn_agent_boot.trn_boot import _ntff_profile_via_ctypes

        h = _ntff_profile_via_ctypes("/opt/axon/libaxon_pjrt.so")
        if h is not None:
            m._hook = h
    except Exception as e:
        print(f"ntff hook install failed: {e}")

    from concourse import bass_utils as bu

    bu.upload_artifacts = lambda tmpdir: tmpdir


def run(cfg: Cfg, inputs: dict, trace: bool = False):
    if trace:
        _ensure_ntff_hook()
    deg_pt, idx16, ids_f32, plan = preprocess(cfg, inputs["edge_index"])
    nc = _get_compiled(cfg, plan)
    in_maps = make_in_maps(
        cfg, inputs["x"], inputs["W1"], inputs["b1"], inputs["W2"], inputs["b2"],
        deg_pt, idx16, ids_f32,
    )
    res = run_bass_kernel_spmd(
        nc, in_maps, core_ids=list(range(cfg.cores)), trace=trace
    )
    c = cfg
    h = np.concatenate(
        [res.results[ci]["out_h"][: c.nsh, : c.dout] for ci in range(c.cores)], axis=0
    )
    ls = np.concatenate(
        [res.results[ci]["out_ls"][: c.nsh, : c.dout] for ci in range(c.cores)], axis=0
    )
    return (h, ls), res


def kernel(**inputs):
    (h, ls), _ = run(Cfg(), inputs)
    return h, ls
